# revision 1
# baseline (speedup 1.0000x reference)
"""Trainium2 Bass kernel for ConvFourierKANLayer.

Computes y = conv2d(cos(x*k), w0) + conv2d(sin(x*k), w1) + bias for
k = 1..10 (G=10 Fourier orders), 3x3 kernel, pad 1, C=64 -> O=128.

Strategy (8 NeuronCores, data-parallel over batch B=16 -> 2 per core):
  - Host pre-transposes fouriercoeffs into 90 lhsT tiles [K=128, O=128]
    where K = (g_parity, c) packs two Fourier orders per matmul, and the
    tile index t enumerates (branch, g_pair, kh, kw).
  - On-chip, x rows are expanded to cos/sin of k*x. The DVE has no fp
    mod, so the argument reduction uses the fp32 magic-number rounding
    trick (only add/sub/mult, all ISA-valid tensor_scalar ops):
        u  = x*(k/2pi) + 16        (positive)
        v  = (u + 2^23) - 2^23     (= round(u), fp32 round-to-nearest)
        w  = u - v                 (in [-0.5, 0.5])
        sin(k*x) = Sin(w * 2pi)    (ScalarE spline, valid on [-pi, pi])
    cos uses u_c = u + 0.25 (phase + pi/2) through the same pipeline.
  - Implicit GEMM: per 8-row output strip, accumulate 90 matmuls
    (branch x g_pair x 3x3 taps) of [K=128]x[O=128] @ [K=128, N=512]
    into one PSUM bank, with float32r (full-rate fp22) arithmetic.
"""

import numpy as np

import concourse.bass as bass
import concourse.mybir as mybir
import concourse.tile as tile
from concourse import bacc
from concourse.bass_utils import run_bass_kernel_spmd

N_CORES = 8
B, C, H, W = 16, 64, 64, 64
O = 128
G = 10
BS = B // N_CORES  # batches per core
HT = 32  # output rows per chunk (4 psum banks of 8 rows each)
NT = 2 * 5 * 9  # weight tiles: branch x g_pair x 3 x 3

PI = float(np.pi)
TWO_PI = float(2 * np.pi)
MAGIC = 8388608.0  # 2^23: fp32 round-to-nearest-integer magic constant

F32 = mybir.dt.float32
F32R = mybir.dt.float32r

_CACHE = {}


def _build_module(reps=1, mmdt="f32r", ht=HT):
    MMDT = {"f32r": F32R, "bf16": mybir.dt.bfloat16, "fp16": mybir.dt.float16}[mmdt]
    nb = ht // 8  # psum banks per chunk
    nc = bacc.Bacc("TRN2", target_bir_lowering=False)
    x_d = nc.dram_tensor("x", [BS, C, H, W], F32, kind="ExternalInput")
    w_d = nc.dram_tensor("w", [128, NT, 128], MMDT, kind="ExternalInput")
    kv_d = nc.dram_tensor("kvec", [128, 5], F32, kind="ExternalInput")
    bias_d = nc.dram_tensor("biasv", [128, 1], F32, kind="ExternalInput")
    y_d = nc.dram_tensor("y", [BS, O, H, W], F32, kind="ExternalOutput")

    mult = mybir.AluOpType.mult
    add = mybir.AluOpType.add
    sin_f = mybir.ActivationFunctionType.Sin

    with tile.TileContext(nc) as tc:
        with (
            tc.tile_pool(name="const", bufs=1) as cpool,
            tc.tile_pool(name="wpool", bufs=1) as wpool,
            tc.tile_pool(name="gen", bufs=2) as gen,
            tc.tile_pool(name="cspool", bufs=3) as cspool,
            tc.tile_pool(name="outp", bufs=3) as outp,
            tc.tile_pool(name="psum", bufs=2, space="PSUM") as psum,
        ):
            wt = wpool.tile([128, NT, 128], MMDT)
            for wi in range(0, NT, 15):
                nc.sync.dma_start(
                    wt[:, wi : wi + 15, :], w_d[:, wi : wi + 15, :]
                )
            kvt = cpool.tile([128, 5], F32)
            nc.sync.dma_start(kvt[:], kv_d[:])
            bt = cpool.tile([128, 1], F32)
            nc.sync.dma_start(bt[:], bias_d[:])
            quarter = cpool.tile([128, 1], F32)
            nc.vector.memset(quarter[:], 0.25)

            for rep in range(reps):
              for b in range(BS):
                for h0 in range(0, H, ht):
                    gr0, gr1 = max(0, h0 - 1), min(H, h0 + ht + 1)
                    l0 = gr0 - (h0 - 1)  # local row index of first real row
                    nrows = gr1 - gr0
                    rs = slice(l0, l0 + nrows)

                    xd = gen.tile([128, ht + 2, W], F32, tag="xdup")
                    nc.sync.dma_start(xd[0:64, rs, :], x_d[b, :, gr0:gr1, :])
                    nc.sync.dma_start(xd[64:128, rs, :], x_d[b, :, gr0:gr1, :])

                    pss = [
                        psum.tile([128, 8, 64], F32, tag=f"ps{bk}",
                                  name=f"ps{bk}_{rep}_{b}_{h0}")
                        for bk in range(nb)
                    ]

                    for j in range(5):
                        # u = x*(k/2pi) + 16 ; v = round(u) ; w = u - v
                        us = gen.tile([128, ht + 2, W], F32, tag="us")
                        nc.vector.tensor_scalar(
                            us[:, rs, :], xd[:, rs, :],
                            kvt[:, j : j + 1], 16.0, mult, add,
                        )
                        uc = gen.tile([128, ht + 2, W], F32, tag="uc")
                        nc.scalar.activation(
                            uc[:, rs, :], us[:, rs, :],
                            mybir.ActivationFunctionType.Identity,
                            bias=quarter[:],
                        )

                        st = cspool.tile([128, ht + 2, W + 2], MMDT, tag="ss")
                        ct = cspool.tile([128, ht + 2, W + 2], MMDT, tag="cs")
                        for u_t, z in ((us, st), (uc, ct)):
                            v_t = gen.tile([128, ht + 2, W], F32, tag="vt", bufs=1)
                            nc.vector.tensor_scalar_add(
                                v_t[:, rs, :], u_t[:, rs, :], MAGIC
                            )
                            nc.vector.tensor_scalar_sub(
                                v_t[:, rs, :], v_t[:, rs, :], MAGIC
                            )
                            w_t = gen.tile([128, ht + 2, W], F32, tag="wt")
                            nc.vector.tensor_sub(
                                w_t[:, rs, :], u_t[:, rs, :], v_t[:, rs, :]
                            )
                            # zero borders (uint32 bitcast: memset can't
                            # encode fp32r), then fill interior with Sin
                            if mmdt == "f32r":
                                u32 = mybir.dt.uint32
                                zb = lambda ap: ap.bitcast(u32)
                            else:
                                zb = lambda ap: ap
                            nc.gpsimd.memset(zb(z[:, :, 0:1]), 0)
                            nc.gpsimd.memset(zb(z[:, :, W + 1 : W + 2]), 0)
                            if l0 == 1:
                                nc.gpsimd.memset(zb(z[:, 0:1, :]), 0)
                            if gr1 == H:
                                nc.gpsimd.memset(
                                    zb(z[:, ht + 1 : ht + 2, :]), 0
                                )
                            nc.scalar.activation(
                                z[:, rs, 1 : W + 1], w_t[:, rs, :], sin_f,
                                scale=TWO_PI,
                            )

                        for br in range(2):
                            src = ct if br == 0 else st
                            for dh in range(3):
                                for dw in range(3):
                                    t_idx = ((br * 5 + j) * 3 + dh) * 3 + dw
                                    for bk in range(nb):
                                        nc.tensor.matmul(
                                            pss[bk][:],
                                            wt[:, t_idx, :],
                                            src[
                                                :,
                                                8 * bk + dh : 8 * bk + dh + 8,
                                                dw : dw + 64,
                                            ],
                                            start=(j == 0 and br == 0
                                                   and dh == 0 and dw == 0),
                                            stop=(j == 4 and br == 1
                                                  and dh == 2 and dw == 2),
                                        )

                    for bk in range(nb):
                        ob = outp.tile([128, 8, 64], F32, tag="ob")
                        nc.vector.tensor_scalar_add(ob[:], pss[bk][:], bt[:, 0:1])
                        nc.sync.dma_start(
                            y_d[b, :, h0 + 8 * bk : h0 + 8 * bk + 8, :], ob[:]
                        )
    nc.finalize()
    return nc


def _get_module(reps=1, mmdt="f32r", ht=HT):
    key = ("nc", reps, mmdt, ht)
    if key not in _CACHE:
        _CACHE[key] = _build_module(reps, mmdt, ht)
    return _CACHE[key]


def _np_mmdt(mmdt):
    import ml_dtypes
    return {"f32r": np.float32, "bf16": ml_dtypes.bfloat16,
            "fp16": np.float16}[mmdt]


def _host_weights(fc, mmdt="f32r"):
    # fc: (2, O, C, kH, kW, G) -> w[p=(gp*64+c), t=(br,j,kh,kw), o]
    W6 = np.transpose(fc, (0, 5, 3, 4, 2, 1))  # (br, g, kh, kw, c, o)
    W6 = W6.reshape(2, 5, 2, 3, 3, 64, 128)  # (br, j, gp, kh, kw, c, o)
    Wt = np.transpose(W6, (0, 1, 3, 4, 2, 5, 6))  # (br, j, kh, kw, gp, c, o)
    Wt = Wt.reshape(NT, 128, 128)
    return np.ascontiguousarray(
        np.transpose(Wt, (1, 0, 2)).astype(_np_mmdt(mmdt))
    )


def _host_kvec():
    kvec = np.zeros((128, 5), np.float32)
    for j in range(5):
        kvec[0:64, j] = (2 * j + 1) / TWO_PI
        kvec[64:128, j] = (2 * j + 2) / TWO_PI
    return kvec


def kernel(x, fouriercoeffs, bias):
    x = np.ascontiguousarray(np.asarray(x, dtype=np.float32))
    fc = np.asarray(fouriercoeffs, dtype=np.float32)
    w_host = _host_weights(fc)
    kvec = _host_kvec()
    biasv = np.ascontiguousarray(
        np.asarray(bias, dtype=np.float32).reshape(128, 1)
    )

    nc = _get_module()
    in_maps = [
        {"x": x[i * BS : (i + 1) * BS], "w": w_host, "kvec": kvec, "biasv": biasv}
        for i in range(N_CORES)
    ]
    res = run_bass_kernel_spmd(nc, in_maps, list(range(N_CORES))).results
    return np.concatenate([res[i]["y"] for i in range(N_CORES)], axis=0)



# revision 16
# speedup vs baseline: 1.0528x; 1.0528x over previous
"""Trainium2 Bass kernel for ConvFourierKANLayer.

Computes y = conv2d(cos(x*k), w0) + conv2d(sin(x*k), w1) + bias for
k = 1..10 (G=10 Fourier orders), 3x3 kernel, pad 1, C=64 -> O=128.

Strategy (8 NeuronCores, data-parallel over batch B=16 -> 2 per core):
  - Implicit GEMM: per 8-row output strip, accumulate taps of
    [K=128]x[O=128] @ [K=128, N=512] into one PSUM bank, where K packs
    (g_parity, c) = two Fourier orders x 64 channels.
  - Mixed precision: most g-pair groups (j) run bf16 matmuls; a tunable
    subset runs fp8(e4m3) with DoubleRow perf mode (cos/sin branches as
    the two contraction sub-planes -> 9 MMs instead of 18 per j).
    Quantization noise is dominated by the fp8 fraction; the fraction is
    chosen so max-err/max|ref| stays well under the 2e-2 gate.
  - Argument range reduction for Sin (valid on [-pi, pi]) uses the fp32
    magic-number rounding trick; the round-to-integer runs on the Scalar
    Engine as Identity(u + 2^23) so the DVE only does one tensor_scalar
    and two scalar_tensor_tensor ops per Fourier pair:
        us  = x*(k/2pi) + 16                       (DVE tensor_scalar, 2x)
        vs  = Identity(us + 2^23)  = round(us)+2^23  (ACT)
        -ws = (vs - 2^23) - us     = round(us)-us    (DVE STT)
        sin/cos: Sin(2pi * -w) = -sin(kx)/-cos(kx)  (ACT; sign folded
        into the host-side weights, which are all negated)
  - All weights are pre-scaled by -1024 (fp8 needs the 2^10 to stay in
    e4m3 normal range; bf16 scaling by a power of 2 is exact); the
    output stage multiplies PSUM by -2^-10 and adds the bias.
"""

import numpy as np

import concourse.bass as bass
import concourse.mybir as mybir
import concourse.tile as tile
from concourse import bacc
from concourse.bass_utils import run_bass_kernel_spmd

N_CORES = 8
B, C, H, W = 16, 64, 64, 64
O = 128
G = 10
BS = B // N_CORES  # batches per core
HT = 32  # output rows per chunk (4 psum banks of 8 rows each)
NT = 2 * 5 * 9  # f32r-legacy weight tiles: branch x g_pair x 3 x 3

PI = float(np.pi)
TWO_PI = float(2 * np.pi)
MAGIC = 8388608.0  # 2^23: fp32 round-to-nearest-integer magic constant
WMAG = 1024.0  # weight pre-scale magnitude (2^10: exact, fp8 normal range)

FP8_JS = (4,)  # g-pair groups run in fp8+DoubleRow (each is 1/5 of work)
RCFG = "tt"  # activation pipeline variant (see _build_mixed)
# per-rcfg signs of what Sin emits on the (cos, sin) planes
_SIGNS = {"tt": (1.0, 1.0), "wrap": (1.0, -1.0), "wrapact": (1.0, -1.0)}

F32 = mybir.dt.float32
F32R = mybir.dt.float32r
BF16 = mybir.dt.bfloat16
FP8 = mybir.dt.float8e4

_CACHE = {}


def _build_mixed(reps=1, fp8_js=FP8_JS, ht=HT, rcfg=RCFG):
    """Mixed bf16 / fp8-DoubleRow kernel.

    rcfg selects the range-reduction pipeline: "tt" all-DVE fused rounds,
    "wrap" cos via ADD_RANGE_WRAP custom op, "wrapact" additionally moves
    the sin round onto the Scalar Engine (Identity + 2^23 bias).
    """
    bf_js = tuple(j for j in range(5) if j not in fp8_js)
    NTB = len(bf_js) * 2 * 9
    NT8 = len(fp8_js) * 9
    nb = ht // 8
    RB = 80  # fp8 row stride (W+2=66 padded) for 16B-aligned plane strides

    nc = bacc.Bacc("TRN2", target_bir_lowering=False)
    x_d = nc.dram_tensor("x", [BS, C, H, W], F32, kind="ExternalInput")
    if NTB:
        wb_d = nc.dram_tensor("wb", [128, NTB, 128], BF16, kind="ExternalInput")
    if NT8:
        w8_d = nc.dram_tensor("w8", [128, NT8, 2, 128], FP8, kind="ExternalInput")
    kv_d = nc.dram_tensor("kvec", [128, 5], F32, kind="ExternalInput")
    bias_d = nc.dram_tensor("biasv", [128, 1], F32, kind="ExternalInput")
    y_d = nc.dram_tensor("y", [BS, O, H, W], F32, kind="ExternalOutput")

    mult = mybir.AluOpType.mult
    add = mybir.AluOpType.add
    sub = mybir.AluOpType.subtract
    sin_f = mybir.ActivationFunctionType.Sin
    ident = mybir.ActivationFunctionType.Identity
    DR = mybir.MatmulPerfMode.DoubleRow

    with tile.TileContext(nc) as tc:
        with (
            tc.tile_pool(name="const", bufs=1) as cpool,
            tc.tile_pool(name="wpool", bufs=1) as wpool,
            tc.tile_pool(name="gen", bufs=2) as gen,
            tc.tile_pool(name="cspool", bufs=3) as cspool,
            tc.tile_pool(name="outp", bufs=3) as outp,
            tc.tile_pool(name="psum", bufs=2, space="PSUM") as psum,
        ):
            if NTB:
                wbt = wpool.tile([128, NTB, 128], BF16)
                for wi in range(0, NTB, 18):
                    nc.sync.dma_start(
                        wbt[:, wi : wi + 18, :], wb_d[:, wi : wi + 18, :]
                    )
            if NT8:
                w8t = wpool.tile([128, NT8, 2, 128], FP8)
                nc.sync.dma_start(w8t[:], w8_d[:])
            kvt = cpool.tile([128, 5], F32)
            nc.sync.dma_start(kvt[:], kv_d[:])
            bt = cpool.tile([128, 1], F32)
            nc.sync.dma_start(bt[:], bias_d[:])
            mg = cpool.tile([128, 1], F32)
            nc.vector.memset(mg[:], MAGIC)
            quarter = cpool.tile([128, 1], F32)
            nc.vector.memset(quarter[:], 0.25)

            # per-j matmul sequencing: bf16 js first, then fp8 js
            n_mm_groups = len(bf_js) * 2 * 9 + len(fp8_js) * 9

            for rep in range(reps):
              for b in range(BS):
                for h0 in range(0, H, ht):
                    gr0, gr1 = max(0, h0 - 1), min(H, h0 + ht + 1)
                    l0 = gr0 - (h0 - 1)  # local row index of first real row
                    nrows = gr1 - gr0
                    rs = slice(l0, l0 + nrows)

                    xd = gen.tile([128, ht + 2, W], F32, tag="xdup")
                    nc.sync.dma_start(xd[0:64, rs, :], x_d[b, :, gr0:gr1, :])
                    nc.sync.dma_start(xd[64:128, rs, :], x_d[b, :, gr0:gr1, :])

                    pss = [
                        psum.tile([128, 8, 64], F32, tag=f"ps{bk}",
                                  name=f"ps{bk}_{rep}_{b}_{h0}")
                        for bk in range(nb)
                    ]

                    mm_idx = [0]  # running tap-group counter for start/stop

                    def gen_ws(j):
                        """Returns (wc, ws) tiles whose Sin(2pi*.) gives
                        SGN_COS*cos(kx) / SGN_SIN*sin(kx)."""
                        us = gen.tile([128, ht + 2, W], F32, tag="us")
                        nc.vector.tensor_scalar(
                            us[:, rs, :], xd[:, rs, :],
                            kvt[:, j : j + 1], 16.0, mult, add,
                        )
                        if rcfg == "wrap":
                            # sin: w_n = round(us)-us ; cos: wrap(w_n+0.25)
                            vs = gen.tile([128, ht + 2, W], F32, tag="vs")
                            nc.vector.tensor_scalar(
                                vs[:, rs, :], us[:, rs, :], MAGIC, MAGIC,
                                add, sub,
                            )
                            ws_n = gen.tile([128, ht + 2, W], F32, tag="ws")
                            nc.vector.tensor_sub(
                                ws_n[:, rs, :], vs[:, rs, :], us[:, rs, :]
                            )
                            wc = gen.tile([128, ht + 2, W], F32, tag="wc")
                            nc.vector.add_range_wrap(
                                wc[:, rs, :], ws_n[:, rs, :],
                                shift=0.25, bound=0.5, period=1.0,
                            )
                            return wc, ws_n
                        if rcfg == "wrapact":
                            vs = gen.tile([128, ht + 2, W], F32, tag="vs")
                            nc.scalar.activation(
                                vs[:, rs, :], us[:, rs, :], ident, bias=mg[:]
                            )
                            ws_n = gen.tile([128, ht + 2, W], F32, tag="ws")
                            nc.vector.scalar_tensor_tensor(
                                ws_n[:, rs, :], vs[:, rs, :], MAGIC,
                                us[:, rs, :], sub, sub,
                            )
                            wc = gen.tile([128, ht + 2, W], F32, tag="wc")
                            nc.vector.add_range_wrap(
                                wc[:, rs, :], ws_n[:, rs, :],
                                shift=0.25, bound=0.5, period=1.0,
                            )
                            return wc, ws_n
                        # rcfg == "tt": baseline-style, fused rounds
                        uc = gen.tile([128, ht + 2, W], F32, tag="uc")
                        nc.scalar.activation(
                            uc[:, rs, :], us[:, rs, :], ident, bias=quarter[:]
                        )
                        ws = gen.tile([128, ht + 2, W], F32, tag="ws")
                        wc = gen.tile([128, ht + 2, W], F32, tag="wc")
                        for u_t, w_t in ((us, ws), (uc, wc)):
                            v_t = gen.tile([128, ht + 2, W], F32, tag="vt",
                                           bufs=1)
                            nc.vector.tensor_scalar(
                                v_t[:, rs, :], u_t[:, rs, :], MAGIC, MAGIC,
                                add, sub,
                            )
                            nc.vector.tensor_sub(
                                w_t[:, rs, :], u_t[:, rs, :], v_t[:, rs, :]
                            )
                        return wc, ws

                    for j in bf_js:
                        wc_n, ws_n = gen_ws(j)
                        jb = bf_js.index(j)
                        ct = cspool.tile([128, ht + 2, W + 2], BF16, tag="cs")
                        st = cspool.tile([128, ht + 2, W + 2], BF16, tag="ss")
                        for z, w_t in ((ct, wc_n), (st, ws_n)):
                            nc.gpsimd.memset(z[:, :, 0:1], 0)
                            nc.gpsimd.memset(z[:, :, W + 1 : W + 2], 0)
                            if l0 == 1:
                                nc.gpsimd.memset(z[:, 0:1, :], 0)
                            if gr1 == H:
                                nc.gpsimd.memset(z[:, ht + 1 : ht + 2, :], 0)
                            nc.scalar.activation(
                                z[:, rs, 1 : W + 1], w_t[:, rs, :], sin_f,
                                scale=TWO_PI,
                            )

                        for br, src in ((0, ct), (1, st)):
                            for dh in range(3):
                                for dw in range(3):
                                    t_idx = ((jb * 2 + br) * 3 + dh) * 3 + dw
                                    i0 = mm_idx[0]
                                    for bk in range(nb):
                                        nc.tensor.matmul(
                                            pss[bk][:],
                                            wbt[:, t_idx, :],
                                            src[
                                                :,
                                                8 * bk + dh : 8 * bk + dh + 8,
                                                dw : dw + 64,
                                            ],
                                            start=(i0 == 0),
                                            stop=(i0 == n_mm_groups - 1),
                                        )
                                    mm_idx[0] += 1

                    for j in fp8_js:
                        wc_n, ws_n = gen_ws(j)
                        j8 = fp8_js.index(j)
                        cst = cspool.tile([128, 2, ht + 2, RB], FP8, tag="c8")
                        for br, w_t in ((0, wc_n), (1, ws_n)):
                            z = cst[:, br]
                            nc.gpsimd.memset(z[:, :, 0:1], 0)
                            nc.gpsimd.memset(z[:, :, 65:66], 0)
                            if l0 == 1:
                                nc.gpsimd.memset(z[:, 0:1, 0:66], 0)
                            if gr1 == H:
                                nc.gpsimd.memset(z[:, ht + 1 : ht + 2, 0:66], 0)
                            nc.scalar.activation(
                                z[:, rs, 1:65], w_t[:, rs, :], sin_f,
                                scale=TWO_PI,
                            )
                        for dh in range(3):
                            for dw in range(3):
                                t_idx = (j8 * 3 + dh) * 3 + dw
                                i0 = mm_idx[0]
                                for bk in range(nb):
                                    nc.tensor.matmul(
                                        pss[bk][:],
                                        w8t[:, t_idx, :, :],
                                        cst[
                                            :, :,
                                            8 * bk + dh : 8 * bk + dh + 8,
                                            dw : dw + 64,
                                        ],
                                        start=(i0 == 0),
                                        stop=(i0 == n_mm_groups - 1),
                                        perf_mode=DR,
                                    )
                                mm_idx[0] += 1

                    assert mm_idx[0] == n_mm_groups

                    for bk in range(nb):
                        ob = outp.tile([128, 8, 64], F32, tag="ob")
                        nc.vector.tensor_scalar(
                            ob[:], pss[bk][:], 1.0 / WMAG, bt[:, 0:1],
                            mult, add,
                        )
                        nc.sync.dma_start(
                            y_d[b, :, h0 + 8 * bk : h0 + 8 * bk + 8, :], ob[:]
                        )
    nc.finalize()
    return nc


def _build_module(reps=1, mmdt="f32r", ht=HT):
    """Legacy single-dtype builder (f32r / bf16 / fp16)."""
    MMDT = {"f32r": F32R, "bf16": BF16, "fp16": mybir.dt.float16}[mmdt]
    nb = ht // 8
    nc = bacc.Bacc("TRN2", target_bir_lowering=False)
    x_d = nc.dram_tensor("x", [BS, C, H, W], F32, kind="ExternalInput")
    w_d = nc.dram_tensor("w", [128, NT, 128], MMDT, kind="ExternalInput")
    kv_d = nc.dram_tensor("kvec", [128, 5], F32, kind="ExternalInput")
    bias_d = nc.dram_tensor("biasv", [128, 1], F32, kind="ExternalInput")
    y_d = nc.dram_tensor("y", [BS, O, H, W], F32, kind="ExternalOutput")

    mult = mybir.AluOpType.mult
    add = mybir.AluOpType.add
    sin_f = mybir.ActivationFunctionType.Sin

    with tile.TileContext(nc) as tc:
        with (
            tc.tile_pool(name="const", bufs=1) as cpool,
            tc.tile_pool(name="wpool", bufs=1) as wpool,
            tc.tile_pool(name="gen", bufs=2) as gen,
            tc.tile_pool(name="cspool", bufs=3) as cspool,
            tc.tile_pool(name="outp", bufs=3) as outp,
            tc.tile_pool(name="psum", bufs=2, space="PSUM") as psum,
        ):
            wt = wpool.tile([128, NT, 128], MMDT)
            for wi in range(0, NT, 15):
                nc.sync.dma_start(
                    wt[:, wi : wi + 15, :], w_d[:, wi : wi + 15, :]
                )
            kvt = cpool.tile([128, 5], F32)
            nc.sync.dma_start(kvt[:], kv_d[:])
            bt = cpool.tile([128, 1], F32)
            nc.sync.dma_start(bt[:], bias_d[:])
            quarter = cpool.tile([128, 1], F32)
            nc.vector.memset(quarter[:], 0.25)

            for rep in range(reps):
              for b in range(BS):
                for h0 in range(0, H, ht):
                    gr0, gr1 = max(0, h0 - 1), min(H, h0 + ht + 1)
                    l0 = gr0 - (h0 - 1)  # local row index of first real row
                    nrows = gr1 - gr0
                    rs = slice(l0, l0 + nrows)

                    xd = gen.tile([128, ht + 2, W], F32, tag="xdup")
                    nc.sync.dma_start(xd[0:64, rs, :], x_d[b, :, gr0:gr1, :])
                    nc.sync.dma_start(xd[64:128, rs, :], x_d[b, :, gr0:gr1, :])

                    pss = [
                        psum.tile([128, 8, 64], F32, tag=f"ps{bk}",
                                  name=f"ps{bk}_{rep}_{b}_{h0}")
                        for bk in range(nb)
                    ]

                    for j in range(5):
                        # u = x*(k/2pi) + 16 ; v = round(u) ; w = u - v
                        us = gen.tile([128, ht + 2, W], F32, tag="us")
                        nc.vector.tensor_scalar(
                            us[:, rs, :], xd[:, rs, :],
                            kvt[:, j : j + 1], 16.0, mult, add,
                        )
                        uc = gen.tile([128, ht + 2, W], F32, tag="uc")
                        nc.scalar.activation(
                            uc[:, rs, :], us[:, rs, :],
                            mybir.ActivationFunctionType.Identity,
                            bias=quarter[:],
                        )

                        st = cspool.tile([128, ht + 2, W + 2], MMDT, tag="ss")
                        ct = cspool.tile([128, ht + 2, W + 2], MMDT, tag="cs")
                        for u_t, z in ((us, st), (uc, ct)):
                            v_t = gen.tile([128, ht + 2, W], F32, tag="vt", bufs=1)
                            nc.vector.tensor_scalar(
                                v_t[:, rs, :], u_t[:, rs, :], MAGIC, MAGIC,
                                mybir.AluOpType.add, mybir.AluOpType.subtract,
                            )
                            w_t = gen.tile([128, ht + 2, W], F32, tag="wt")
                            nc.vector.tensor_sub(
                                w_t[:, rs, :], u_t[:, rs, :], v_t[:, rs, :]
                            )
                            # zero borders (uint32 bitcast: memset can't
                            # encode fp32r), then fill interior with Sin
                            if mmdt == "f32r":
                                u32 = mybir.dt.uint32
                                zb = lambda ap: ap.bitcast(u32)
                            else:
                                zb = lambda ap: ap
                            nc.gpsimd.memset(zb(z[:, :, 0:1]), 0)
                            nc.gpsimd.memset(zb(z[:, :, W + 1 : W + 2]), 0)
                            if l0 == 1:
                                nc.gpsimd.memset(zb(z[:, 0:1, :]), 0)
                            if gr1 == H:
                                nc.gpsimd.memset(
                                    zb(z[:, ht + 1 : ht + 2, :]), 0
                                )
                            nc.scalar.activation(
                                z[:, rs, 1 : W + 1], w_t[:, rs, :], sin_f,
                                scale=TWO_PI,
                            )

                        for br in range(2):
                            src = ct if br == 0 else st
                            for dh in range(3):
                                for dw in range(3):
                                    t_idx = ((br * 5 + j) * 3 + dh) * 3 + dw
                                    for bk in range(nb):
                                        nc.tensor.matmul(
                                            pss[bk][:],
                                            wt[:, t_idx, :],
                                            src[
                                                :,
                                                8 * bk + dh : 8 * bk + dh + 8,
                                                dw : dw + 64,
                                            ],
                                            start=(j == 0 and br == 0
                                                   and dh == 0 and dw == 0),
                                            stop=(j == 4 and br == 1
                                                  and dh == 2 and dw == 2),
                                        )

                    for bk in range(nb):
                        ob = outp.tile([128, 8, 64], F32, tag="ob")
                        nc.vector.tensor_scalar_add(ob[:], pss[bk][:], bt[:, 0:1])
                        nc.sync.dma_start(
                            y_d[b, :, h0 + 8 * bk : h0 + 8 * bk + 8, :], ob[:]
                        )
    nc.finalize()
    return nc


def _get_module(reps=1, mmdt="mix", ht=HT, fp8_js=FP8_JS, rcfg=RCFG):
    if mmdt == "mix":
        key = ("mix", reps, ht, fp8_js, rcfg)
        if key not in _CACHE:
            _CACHE[key] = _build_mixed(reps, fp8_js, ht, rcfg)
        return _CACHE[key]
    key = ("nc", reps, mmdt, ht)
    if key not in _CACHE:
        _CACHE[key] = _build_module(reps, mmdt, ht)
    return _CACHE[key]


def _np_mmdt(mmdt):
    import ml_dtypes
    return {"f32r": np.float32, "bf16": ml_dtypes.bfloat16,
            "fp16": np.float16}[mmdt]


def _weight_planes(fc):
    # fc: (2, O, C, kH, kW, G) -> (br, j, kh, kw, p=(gp*64+c), o) fp32
    W6 = np.transpose(fc, (0, 5, 3, 4, 2, 1))  # (br, g, kh, kw, c, o)
    W6 = W6.reshape(2, 5, 2, 3, 3, 64, 128)  # (br, j, gp, kh, kw, c, o)
    Wt = np.transpose(W6, (0, 1, 3, 4, 2, 5, 6))  # (br, j, kh, kw, gp, c, o)
    return Wt.reshape(2, 5, 3, 3, 128, 128)


def _host_weights(fc, mmdt="f32r"):
    # legacy single-dtype layout: w[p, t=(br,j,kh,kw), o]
    Wt = _weight_planes(fc).reshape(NT, 128, 128)
    return np.ascontiguousarray(
        np.transpose(Wt, (1, 0, 2)).astype(_np_mmdt(mmdt))
    )


def _host_weights_mixed(fc, fp8_js=FP8_JS, rcfg=RCFG):
    import ml_dtypes
    sc, ss = _SIGNS[rcfg]
    Wp = _weight_planes(fc) * WMAG  # (br, j, kh, kw, p, o)
    Wp = Wp * np.array([sc, ss]).reshape(2, 1, 1, 1, 1, 1)
    bf_js = tuple(j for j in range(5) if j not in fp8_js)
    out = {}
    if bf_js:
        wb = Wp[:, bf_js]  # (br, nj, kh, kw, p, o)
        # tile order: ((jb*2 + br)*3 + kh)*3 + kw
        wb = np.transpose(wb, (1, 0, 2, 3, 4, 5))  # (nj, br, kh, kw, p, o)
        wb = wb.reshape(len(bf_js) * 2 * 9, 128, 128)
        out["wb"] = np.ascontiguousarray(
            np.transpose(wb, (1, 0, 2)).astype(ml_dtypes.bfloat16)
        )
    if fp8_js:
        w8 = Wp[:, fp8_js]  # (br, nj, kh, kw, p, o)
        w8 = np.transpose(w8, (1, 2, 3, 4, 0, 5))  # (nj, kh, kw, p, br, o)
        w8 = w8.reshape(len(fp8_js) * 9, 128, 2, 128)
        out["w8"] = np.ascontiguousarray(
            np.transpose(w8, (1, 0, 2, 3)).astype(ml_dtypes.float8_e4m3)
        )
    return out


def _host_kvec():
    kvec = np.zeros((128, 5), np.float32)
    for j in range(5):
        kvec[0:64, j] = (2 * j + 1) / TWO_PI
        kvec[64:128, j] = (2 * j + 2) / TWO_PI
    return kvec


def _host_inputs(x, fouriercoeffs, bias, mmdt="mix", fp8_js=FP8_JS, rcfg=RCFG):
    x = np.ascontiguousarray(np.asarray(x, dtype=np.float32))
    fc = np.asarray(fouriercoeffs, dtype=np.float32)
    base = {
        "kvec": _host_kvec(),
        "biasv": np.ascontiguousarray(
            np.asarray(bias, dtype=np.float32).reshape(128, 1)
        ),
    }
    if mmdt == "mix":
        base.update(_host_weights_mixed(fc, fp8_js, rcfg))
    else:
        base["w"] = _host_weights(fc, mmdt)
    return x, base


def kernel(x, fouriercoeffs, bias):
    x, base = _host_inputs(x, fouriercoeffs, bias, "mix", FP8_JS, RCFG)
    nc = _get_module(1, "mix", fp8_js=FP8_JS, rcfg=RCFG)
    in_maps = [
        dict(base, x=x[i * BS : (i + 1) * BS]) for i in range(N_CORES)
    ]
    res = run_bass_kernel_spmd(nc, in_maps, list(range(N_CORES))).results
    return np.concatenate([res[i]["y"] for i in range(N_CORES)], axis=0)


# revision 17
# speedup vs baseline: 1.6236x; 1.5422x over previous
"""Trainium2 Bass kernel for ConvFourierKANLayer.

Computes y = conv2d(cos(x*k), w0) + conv2d(sin(x*k), w1) + bias for
k = 1..10 (G=10 Fourier orders), 3x3 kernel, pad 1, C=64 -> O=128.

Strategy (8 NeuronCores, data-parallel over batch B=16 -> 2 per core):
  - Implicit GEMM: per 8-row output strip, accumulate taps of
    [K=128]x[O=128] @ [K=128, N=512] into one PSUM bank, where K packs
    (g_parity, c) = two Fourier orders x 64 channels.
  - Mixed precision: most g-pair groups (j) run bf16 matmuls; a tunable
    subset runs fp8(e4m3) with DoubleRow perf mode (cos/sin branches as
    the two contraction sub-planes -> 9 MMs instead of 18 per j).
    Quantization noise is dominated by the fp8 fraction; the fraction is
    chosen so max-err/max|ref| stays well under the 2e-2 gate.
  - Argument range reduction for Sin (valid on [-pi, pi]) uses the fp32
    magic-number rounding trick; the round-to-integer runs on the Scalar
    Engine as Identity(u + 2^23) so the DVE only does one tensor_scalar
    and two scalar_tensor_tensor ops per Fourier pair:
        us  = x*(k/2pi) + 16                       (DVE tensor_scalar, 2x)
        vs  = Identity(us + 2^23)  = round(us)+2^23  (ACT)
        -ws = (vs - 2^23) - us     = round(us)-us    (DVE STT)
        sin/cos: Sin(2pi * -w) = -sin(kx)/-cos(kx)  (ACT; sign folded
        into the host-side weights, which are all negated)
  - All weights are pre-scaled by -1024 (fp8 needs the 2^10 to stay in
    e4m3 normal range; bf16 scaling by a power of 2 is exact); the
    output stage multiplies PSUM by -2^-10 and adds the bias.
"""

import numpy as np

import concourse.bass as bass
import concourse.mybir as mybir
import concourse.tile as tile
from concourse import bacc
from concourse.bass_utils import run_bass_kernel_spmd

N_CORES = 8
B, C, H, W = 16, 64, 64, 64
O = 128
G = 10
BS = B // N_CORES  # batches per core
HT = 32  # output rows per chunk (4 psum banks of 8 rows each)
NT = 2 * 5 * 9  # f32r-legacy weight tiles: branch x g_pair x 3 x 3

PI = float(np.pi)
TWO_PI = float(2 * np.pi)
MAGIC = 8388608.0  # 2^23: fp32 round-to-nearest-integer magic constant
WMAG = 1024.0  # weight pre-scale magnitude (2^10: exact, fp8 normal range)

FP8_JS = (2, 3, 4)  # g-pair groups run in fp8+DoubleRow (each 1/5 of work)
NCH = 2  # fp8 weight chains: 2 = hi+lo split (kills fp8 weight-quant noise)
RCFG = "wrapact"  # activation pipeline variant (see _build_mixed)
# per-rcfg signs of what Sin emits on the (cos, sin) planes
_SIGNS = {"tt": (1.0, 1.0), "wrap": (1.0, -1.0), "wrapact": (1.0, -1.0)}

F32 = mybir.dt.float32
F32R = mybir.dt.float32r
BF16 = mybir.dt.bfloat16
FP8 = mybir.dt.float8e4

_CACHE = {}


def _build_mixed(reps=1, fp8_js=FP8_JS, ht=HT, rcfg=RCFG, nch=NCH):
    """Mixed bf16 / fp8-DoubleRow kernel.

    rcfg selects the range-reduction pipeline: "tt" all-DVE fused rounds,
    "wrap" cos via ADD_RANGE_WRAP custom op, "wrapact" additionally moves
    the sin round onto the Scalar Engine (Identity + 2^23 bias).
    """
    bf_js = tuple(j for j in range(5) if j not in fp8_js)
    NTB = len(bf_js) * 2 * 9
    NT8 = len(fp8_js) * nch * 9
    nb = ht // 8
    RB = 80  # fp8 row stride (W+2=66 padded) for 16B-aligned plane strides

    nc = bacc.Bacc("TRN2", target_bir_lowering=False)
    x_d = nc.dram_tensor("x", [BS, C, H, W], F32, kind="ExternalInput")
    if NTB:
        wb_d = nc.dram_tensor("wb", [128, NTB, 128], BF16, kind="ExternalInput")
    if NT8:
        w8_d = nc.dram_tensor("w8", [128, NT8, 2, 128], FP8, kind="ExternalInput")
    kv_d = nc.dram_tensor("kvec", [128, 5], F32, kind="ExternalInput")
    bias_d = nc.dram_tensor("biasv", [128, 1], F32, kind="ExternalInput")
    y_d = nc.dram_tensor("y", [BS, O, H, W], F32, kind="ExternalOutput")

    mult = mybir.AluOpType.mult
    add = mybir.AluOpType.add
    sub = mybir.AluOpType.subtract
    sin_f = mybir.ActivationFunctionType.Sin
    ident = mybir.ActivationFunctionType.Identity
    DR = mybir.MatmulPerfMode.DoubleRow

    with tile.TileContext(nc) as tc:
        with (
            tc.tile_pool(name="const", bufs=1) as cpool,
            tc.tile_pool(name="wpool", bufs=1) as wpool,
            tc.tile_pool(name="gen", bufs=2) as gen,
            tc.tile_pool(name="cspool", bufs=3) as cspool,
            tc.tile_pool(name="outp", bufs=3) as outp,
            tc.tile_pool(name="psum", bufs=2, space="PSUM") as psum,
        ):
            if NTB:
                wbt = wpool.tile([128, NTB, 128], BF16)
                for wi in range(0, NTB, 18):
                    nc.sync.dma_start(
                        wbt[:, wi : wi + 18, :], wb_d[:, wi : wi + 18, :]
                    )
            if NT8:
                w8t = wpool.tile([128, NT8, 2, 128], FP8)
                nc.sync.dma_start(w8t[:], w8_d[:])
            kvt = cpool.tile([128, 5], F32)
            nc.sync.dma_start(kvt[:], kv_d[:])
            bt = cpool.tile([128, 1], F32)
            nc.sync.dma_start(bt[:], bias_d[:])
            mg = cpool.tile([128, 1], F32)
            nc.vector.memset(mg[:], MAGIC)
            quarter = cpool.tile([128, 1], F32)
            nc.vector.memset(quarter[:], 0.25)

            # per-j matmul sequencing: bf16 js first, then fp8 js
            n_mm_groups = len(bf_js) * 2 * 9 + len(fp8_js) * nch * 9

            for rep in range(reps):
              for b in range(BS):
                for h0 in range(0, H, ht):
                    gr0, gr1 = max(0, h0 - 1), min(H, h0 + ht + 1)
                    l0 = gr0 - (h0 - 1)  # local row index of first real row
                    nrows = gr1 - gr0
                    rs = slice(l0, l0 + nrows)

                    xd = gen.tile([128, ht + 2, W], F32, tag="xdup")
                    nc.sync.dma_start(xd[0:64, rs, :], x_d[b, :, gr0:gr1, :])
                    nc.sync.dma_start(xd[64:128, rs, :], x_d[b, :, gr0:gr1, :])

                    pss = [
                        psum.tile([128, 8, 64], F32, tag=f"ps{bk}",
                                  name=f"ps{bk}_{rep}_{b}_{h0}")
                        for bk in range(nb)
                    ]

                    mm_idx = [0]  # running tap-group counter for start/stop

                    def gen_ws(j):
                        """Returns (wc, ws) tiles whose Sin(2pi*.) gives
                        SGN_COS*cos(kx) / SGN_SIN*sin(kx)."""
                        us = gen.tile([128, ht + 2, W], F32, tag="us")
                        nc.vector.tensor_scalar(
                            us[:, rs, :], xd[:, rs, :],
                            kvt[:, j : j + 1], 16.0, mult, add,
                        )
                        if rcfg == "wrap":
                            # sin: w_n = round(us)-us ; cos: wrap(w_n+0.25)
                            vs = gen.tile([128, ht + 2, W], F32, tag="vs")
                            nc.vector.tensor_scalar(
                                vs[:, rs, :], us[:, rs, :], MAGIC, MAGIC,
                                add, sub,
                            )
                            ws_n = gen.tile([128, ht + 2, W], F32, tag="ws")
                            nc.vector.tensor_sub(
                                ws_n[:, rs, :], vs[:, rs, :], us[:, rs, :]
                            )
                            wc = gen.tile([128, ht + 2, W], F32, tag="wc")
                            nc.vector.add_range_wrap(
                                wc[:, rs, :], ws_n[:, rs, :],
                                shift=0.25, bound=0.5, period=1.0,
                            )
                            return wc, ws_n
                        if rcfg == "wrapact":
                            vs = gen.tile([128, ht + 2, W], F32, tag="vs")
                            nc.scalar.activation(
                                vs[:, rs, :], us[:, rs, :], ident, bias=mg[:]
                            )
                            ws_n = gen.tile([128, ht + 2, W], F32, tag="ws")
                            nc.vector.scalar_tensor_tensor(
                                ws_n[:, rs, :], vs[:, rs, :], MAGIC,
                                us[:, rs, :], sub, sub,
                            )
                            wc = gen.tile([128, ht + 2, W], F32, tag="wc")
                            nc.vector.add_range_wrap(
                                wc[:, rs, :], ws_n[:, rs, :],
                                shift=0.25, bound=0.5, period=1.0,
                            )
                            return wc, ws_n
                        # rcfg == "tt": baseline-style, fused rounds
                        uc = gen.tile([128, ht + 2, W], F32, tag="uc")
                        nc.scalar.activation(
                            uc[:, rs, :], us[:, rs, :], ident, bias=quarter[:]
                        )
                        ws = gen.tile([128, ht + 2, W], F32, tag="ws")
                        wc = gen.tile([128, ht + 2, W], F32, tag="wc")
                        for u_t, w_t in ((us, ws), (uc, wc)):
                            v_t = gen.tile([128, ht + 2, W], F32, tag="vt",
                                           bufs=1)
                            nc.vector.tensor_scalar(
                                v_t[:, rs, :], u_t[:, rs, :], MAGIC, MAGIC,
                                add, sub,
                            )
                            nc.vector.tensor_sub(
                                w_t[:, rs, :], u_t[:, rs, :], v_t[:, rs, :]
                            )
                        return wc, ws

                    for j in bf_js:
                        wc_n, ws_n = gen_ws(j)
                        jb = bf_js.index(j)
                        ct = cspool.tile([128, ht + 2, W + 2], BF16, tag="cs")
                        st = cspool.tile([128, ht + 2, W + 2], BF16, tag="ss")
                        for z, w_t in ((ct, wc_n), (st, ws_n)):
                            nc.gpsimd.memset(z[:, :, 0:1], 0)
                            nc.gpsimd.memset(z[:, :, W + 1 : W + 2], 0)
                            if l0 == 1:
                                nc.gpsimd.memset(z[:, 0:1, :], 0)
                            if gr1 == H:
                                nc.gpsimd.memset(z[:, ht + 1 : ht + 2, :], 0)
                            nc.scalar.activation(
                                z[:, rs, 1 : W + 1], w_t[:, rs, :], sin_f,
                                scale=TWO_PI,
                            )

                        for br, src in ((0, ct), (1, st)):
                            for dh in range(3):
                                for dw in range(3):
                                    t_idx = ((jb * 2 + br) * 3 + dh) * 3 + dw
                                    i0 = mm_idx[0]
                                    for bk in range(nb):
                                        nc.tensor.matmul(
                                            pss[bk][:],
                                            wbt[:, t_idx, :],
                                            src[
                                                :,
                                                8 * bk + dh : 8 * bk + dh + 8,
                                                dw : dw + 64,
                                            ],
                                            start=(i0 == 0),
                                            stop=(i0 == n_mm_groups - 1),
                                        )
                                    mm_idx[0] += 1

                    for j in fp8_js:
                        wc_n, ws_n = gen_ws(j)
                        j8 = fp8_js.index(j)
                        cst = cspool.tile([128, 2, ht + 2, RB], FP8, tag="c8")
                        for br, w_t in ((0, wc_n), (1, ws_n)):
                            z = cst[:, br]
                            nc.gpsimd.memset(z[:, :, 0:1], 0)
                            nc.gpsimd.memset(z[:, :, 65:66], 0)
                            if l0 == 1:
                                nc.gpsimd.memset(z[:, 0:1, 0:66], 0)
                            if gr1 == H:
                                nc.gpsimd.memset(z[:, ht + 1 : ht + 2, 0:66], 0)
                            nc.scalar.activation(
                                z[:, rs, 1:65], w_t[:, rs, :], sin_f,
                                scale=TWO_PI,
                            )
                        for ch in range(nch):
                            for dh in range(3):
                                for dw in range(3):
                                    t_idx = ((j8 * nch + ch) * 3 + dh) * 3 + dw
                                    i0 = mm_idx[0]
                                    for bk in range(nb):
                                        nc.tensor.matmul(
                                            pss[bk][:],
                                            w8t[:, t_idx, :, :],
                                            cst[
                                                :, :,
                                                8 * bk + dh : 8 * bk + dh + 8,
                                                dw : dw + 64,
                                            ],
                                            start=(i0 == 0),
                                            stop=(i0 == n_mm_groups - 1),
                                            perf_mode=DR,
                                        )
                                    mm_idx[0] += 1

                    assert mm_idx[0] == n_mm_groups

                    for bk in range(nb):
                        ob = outp.tile([128, 8, 64], F32, tag="ob")
                        nc.vector.tensor_scalar(
                            ob[:], pss[bk][:], 1.0 / WMAG, bt[:, 0:1],
                            mult, add,
                        )
                        nc.sync.dma_start(
                            y_d[b, :, h0 + 8 * bk : h0 + 8 * bk + 8, :], ob[:]
                        )
    nc.finalize()
    return nc


def _build_module(reps=1, mmdt="f32r", ht=HT):
    """Legacy single-dtype builder (f32r / bf16 / fp16)."""
    MMDT = {"f32r": F32R, "bf16": BF16, "fp16": mybir.dt.float16}[mmdt]
    nb = ht // 8
    nc = bacc.Bacc("TRN2", target_bir_lowering=False)
    x_d = nc.dram_tensor("x", [BS, C, H, W], F32, kind="ExternalInput")
    w_d = nc.dram_tensor("w", [128, NT, 128], MMDT, kind="ExternalInput")
    kv_d = nc.dram_tensor("kvec", [128, 5], F32, kind="ExternalInput")
    bias_d = nc.dram_tensor("biasv", [128, 1], F32, kind="ExternalInput")
    y_d = nc.dram_tensor("y", [BS, O, H, W], F32, kind="ExternalOutput")

    mult = mybir.AluOpType.mult
    add = mybir.AluOpType.add
    sin_f = mybir.ActivationFunctionType.Sin

    with tile.TileContext(nc) as tc:
        with (
            tc.tile_pool(name="const", bufs=1) as cpool,
            tc.tile_pool(name="wpool", bufs=1) as wpool,
            tc.tile_pool(name="gen", bufs=2) as gen,
            tc.tile_pool(name="cspool", bufs=3) as cspool,
            tc.tile_pool(name="outp", bufs=3) as outp,
            tc.tile_pool(name="psum", bufs=2, space="PSUM") as psum,
        ):
            wt = wpool.tile([128, NT, 128], MMDT)
            for wi in range(0, NT, 15):
                nc.sync.dma_start(
                    wt[:, wi : wi + 15, :], w_d[:, wi : wi + 15, :]
                )
            kvt = cpool.tile([128, 5], F32)
            nc.sync.dma_start(kvt[:], kv_d[:])
            bt = cpool.tile([128, 1], F32)
            nc.sync.dma_start(bt[:], bias_d[:])
            quarter = cpool.tile([128, 1], F32)
            nc.vector.memset(quarter[:], 0.25)

            for rep in range(reps):
              for b in range(BS):
                for h0 in range(0, H, ht):
                    gr0, gr1 = max(0, h0 - 1), min(H, h0 + ht + 1)
                    l0 = gr0 - (h0 - 1)  # local row index of first real row
                    nrows = gr1 - gr0
                    rs = slice(l0, l0 + nrows)

                    xd = gen.tile([128, ht + 2, W], F32, tag="xdup")
                    nc.sync.dma_start(xd[0:64, rs, :], x_d[b, :, gr0:gr1, :])
                    nc.sync.dma_start(xd[64:128, rs, :], x_d[b, :, gr0:gr1, :])

                    pss = [
                        psum.tile([128, 8, 64], F32, tag=f"ps{bk}",
                                  name=f"ps{bk}_{rep}_{b}_{h0}")
                        for bk in range(nb)
                    ]

                    for j in range(5):
                        # u = x*(k/2pi) + 16 ; v = round(u) ; w = u - v
                        us = gen.tile([128, ht + 2, W], F32, tag="us")
                        nc.vector.tensor_scalar(
                            us[:, rs, :], xd[:, rs, :],
                            kvt[:, j : j + 1], 16.0, mult, add,
                        )
                        uc = gen.tile([128, ht + 2, W], F32, tag="uc")
                        nc.scalar.activation(
                            uc[:, rs, :], us[:, rs, :],
                            mybir.ActivationFunctionType.Identity,
                            bias=quarter[:],
                        )

                        st = cspool.tile([128, ht + 2, W + 2], MMDT, tag="ss")
                        ct = cspool.tile([128, ht + 2, W + 2], MMDT, tag="cs")
                        for u_t, z in ((us, st), (uc, ct)):
                            v_t = gen.tile([128, ht + 2, W], F32, tag="vt", bufs=1)
                            nc.vector.tensor_scalar(
                                v_t[:, rs, :], u_t[:, rs, :], MAGIC, MAGIC,
                                mybir.AluOpType.add, mybir.AluOpType.subtract,
                            )
                            w_t = gen.tile([128, ht + 2, W], F32, tag="wt")
                            nc.vector.tensor_sub(
                                w_t[:, rs, :], u_t[:, rs, :], v_t[:, rs, :]
                            )
                            # zero borders (uint32 bitcast: memset can't
                            # encode fp32r), then fill interior with Sin
                            if mmdt == "f32r":
                                u32 = mybir.dt.uint32
                                zb = lambda ap: ap.bitcast(u32)
                            else:
                                zb = lambda ap: ap
                            nc.gpsimd.memset(zb(z[:, :, 0:1]), 0)
                            nc.gpsimd.memset(zb(z[:, :, W + 1 : W + 2]), 0)
                            if l0 == 1:
                                nc.gpsimd.memset(zb(z[:, 0:1, :]), 0)
                            if gr1 == H:
                                nc.gpsimd.memset(
                                    zb(z[:, ht + 1 : ht + 2, :]), 0
                                )
                            nc.scalar.activation(
                                z[:, rs, 1 : W + 1], w_t[:, rs, :], sin_f,
                                scale=TWO_PI,
                            )

                        for br in range(2):
                            src = ct if br == 0 else st
                            for dh in range(3):
                                for dw in range(3):
                                    t_idx = ((br * 5 + j) * 3 + dh) * 3 + dw
                                    for bk in range(nb):
                                        nc.tensor.matmul(
                                            pss[bk][:],
                                            wt[:, t_idx, :],
                                            src[
                                                :,
                                                8 * bk + dh : 8 * bk + dh + 8,
                                                dw : dw + 64,
                                            ],
                                            start=(j == 0 and br == 0
                                                   and dh == 0 and dw == 0),
                                            stop=(j == 4 and br == 1
                                                  and dh == 2 and dw == 2),
                                        )

                    for bk in range(nb):
                        ob = outp.tile([128, 8, 64], F32, tag="ob")
                        nc.vector.tensor_scalar_add(ob[:], pss[bk][:], bt[:, 0:1])
                        nc.sync.dma_start(
                            y_d[b, :, h0 + 8 * bk : h0 + 8 * bk + 8, :], ob[:]
                        )
    nc.finalize()
    return nc


def _get_module(reps=1, mmdt="mix", ht=HT, fp8_js=FP8_JS, rcfg=RCFG, nch=NCH):
    if mmdt == "mix":
        key = ("mix", reps, ht, fp8_js, rcfg, nch)
        if key not in _CACHE:
            _CACHE[key] = _build_mixed(reps, fp8_js, ht, rcfg, nch)
        return _CACHE[key]
    key = ("nc", reps, mmdt, ht)
    if key not in _CACHE:
        _CACHE[key] = _build_module(reps, mmdt, ht)
    return _CACHE[key]


def _np_mmdt(mmdt):
    import ml_dtypes
    return {"f32r": np.float32, "bf16": ml_dtypes.bfloat16,
            "fp16": np.float16}[mmdt]


def _weight_planes(fc):
    # fc: (2, O, C, kH, kW, G) -> (br, j, kh, kw, p=(gp*64+c), o) fp32
    W6 = np.transpose(fc, (0, 5, 3, 4, 2, 1))  # (br, g, kh, kw, c, o)
    W6 = W6.reshape(2, 5, 2, 3, 3, 64, 128)  # (br, j, gp, kh, kw, c, o)
    Wt = np.transpose(W6, (0, 1, 3, 4, 2, 5, 6))  # (br, j, kh, kw, gp, c, o)
    return Wt.reshape(2, 5, 3, 3, 128, 128)


def _host_weights(fc, mmdt="f32r"):
    # legacy single-dtype layout: w[p, t=(br,j,kh,kw), o]
    Wt = _weight_planes(fc).reshape(NT, 128, 128)
    return np.ascontiguousarray(
        np.transpose(Wt, (1, 0, 2)).astype(_np_mmdt(mmdt))
    )


def _host_weights_mixed(fc, fp8_js=FP8_JS, rcfg=RCFG, nch=NCH):
    import ml_dtypes
    sc, ss = _SIGNS[rcfg]
    Wp = _weight_planes(fc) * WMAG  # (br, j, kh, kw, p, o)
    Wp = Wp * np.array([sc, ss]).reshape(2, 1, 1, 1, 1, 1)
    bf_js = tuple(j for j in range(5) if j not in fp8_js)
    out = {}
    if bf_js:
        wb = Wp[:, bf_js]  # (br, nj, kh, kw, p, o)
        # tile order: ((jb*2 + br)*3 + kh)*3 + kw
        wb = np.transpose(wb, (1, 0, 2, 3, 4, 5))  # (nj, br, kh, kw, p, o)
        wb = wb.reshape(len(bf_js) * 2 * 9, 128, 128)
        out["wb"] = np.ascontiguousarray(
            np.transpose(wb, (1, 0, 2)).astype(ml_dtypes.bfloat16)
        )
    if fp8_js:
        w8 = Wp[:, fp8_js]  # (br, nj, kh, kw, p, o)
        w8 = np.transpose(w8, (1, 2, 3, 4, 0, 5))  # (nj, kh, kw, p, br, o)
        w8 = w8.astype(np.float32)
        hi = w8.astype(ml_dtypes.float8_e4m3)
        if nch == 2:
            lo = (w8 - hi.astype(np.float32)).astype(ml_dtypes.float8_e4m3)
            w8q = np.stack([hi, lo], axis=1)  # (nj, nch, kh, kw, p, br, o)
        else:
            w8q = hi[:, None]
        w8q = w8q.reshape(len(fp8_js) * nch * 9, 128, 2, 128)
        out["w8"] = np.ascontiguousarray(np.transpose(w8q, (1, 0, 2, 3)))
    return out


def _host_kvec():
    kvec = np.zeros((128, 5), np.float32)
    for j in range(5):
        kvec[0:64, j] = (2 * j + 1) / TWO_PI
        kvec[64:128, j] = (2 * j + 2) / TWO_PI
    return kvec


def _host_inputs(x, fouriercoeffs, bias, mmdt="mix", fp8_js=FP8_JS, rcfg=RCFG):
    x = np.ascontiguousarray(np.asarray(x, dtype=np.float32))
    fc = np.asarray(fouriercoeffs, dtype=np.float32)
    base = {
        "kvec": _host_kvec(),
        "biasv": np.ascontiguousarray(
            np.asarray(bias, dtype=np.float32).reshape(128, 1)
        ),
    }
    if mmdt == "mix":
        base.update(_host_weights_mixed(fc, fp8_js, rcfg))
    else:
        base["w"] = _host_weights(fc, mmdt)
    return x, base


def kernel(x, fouriercoeffs, bias):
    x, base = _host_inputs(x, fouriercoeffs, bias, "mix", FP8_JS, RCFG)
    nc = _get_module(1, "mix", fp8_js=FP8_JS, rcfg=RCFG)
    in_maps = [
        dict(base, x=x[i * BS : (i + 1) * BS]) for i in range(N_CORES)
    ]
    res = run_bass_kernel_spmd(nc, in_maps, list(range(N_CORES))).results
    return np.concatenate([res[i]["y"] for i in range(N_CORES)], axis=0)


# revision 18
# speedup vs baseline: 2.0093x; 1.2375x over previous
"""Trainium2 Bass kernel for ConvFourierKANLayer.

Computes y = conv2d(cos(x*k), w0) + conv2d(sin(x*k), w1) + bias for
k = 1..10 (G=10 Fourier orders), 3x3 kernel, pad 1, C=64 -> O=128.

Strategy (8 NeuronCores, data-parallel over batch B=16 -> 2 per core):
  - Implicit GEMM: per 8-row output strip, accumulate taps of
    [K=128]x[O=128] @ [K=128, N=512] into one PSUM bank, where K packs
    (g_parity, c) = two Fourier orders x 64 channels.
  - Mixed precision: most g-pair groups (j) run bf16 matmuls; a tunable
    subset runs fp8(e4m3) with DoubleRow perf mode (cos/sin branches as
    the two contraction sub-planes -> 9 MMs instead of 18 per j).
    Quantization noise is dominated by the fp8 fraction; the fraction is
    chosen so max-err/max|ref| stays well under the 2e-2 gate.
  - Argument range reduction for Sin (valid on [-pi, pi]) uses the fp32
    magic-number rounding trick; the round-to-integer runs on the Scalar
    Engine as Identity(u + 2^23) so the DVE only does one tensor_scalar
    and two scalar_tensor_tensor ops per Fourier pair:
        us  = x*(k/2pi) + 16                       (DVE tensor_scalar, 2x)
        vs  = Identity(us + 2^23)  = round(us)+2^23  (ACT)
        -ws = (vs - 2^23) - us     = round(us)-us    (DVE STT)
        sin/cos: Sin(2pi * -w) = -sin(kx)/-cos(kx)  (ACT; sign folded
        into the host-side weights, which are all negated)
  - All weights are pre-scaled by -1024 (fp8 needs the 2^10 to stay in
    e4m3 normal range; bf16 scaling by a power of 2 is exact); the
    output stage multiplies PSUM by -2^-10 and adds the bias.
"""

import numpy as np

import concourse.bass as bass
import concourse.mybir as mybir
import concourse.tile as tile
from concourse import bacc
from concourse.bass_utils import run_bass_kernel_spmd

N_CORES = 8
B, C, H, W = 16, 64, 64, 64
O = 128
G = 10
BS = B // N_CORES  # batches per core
HT = 32  # output rows per chunk (4 psum banks of 8 rows each)
NT = 2 * 5 * 9  # f32r-legacy weight tiles: branch x g_pair x 3 x 3

PI = float(np.pi)
TWO_PI = float(2 * np.pi)
MAGIC = 8388608.0  # 2^23: fp32 round-to-nearest-integer magic constant
WMAG = 1024.0  # weight pre-scale magnitude (2^10: exact, fp8 normal range)

FP8_JS = (4,)  # g-pair groups run in fp8+DoubleRow (each 1/5 of work)
NCH = 1  # fp8 weight chains (2 = hi+lo split, same cost as bf16 -> unused)
RCFG = "wrapact"  # activation pipeline variant (see _build_mixed)
# per-rcfg signs of what Sin emits on the (cos, sin) planes
_SIGNS = {"tt": (1.0, 1.0), "wrap": (1.0, -1.0), "wrapact": (1.0, -1.0)}

F32 = mybir.dt.float32
F32R = mybir.dt.float32r
BF16 = mybir.dt.bfloat16
FP8 = mybir.dt.float8e4

_CACHE = {}


def _build_mixed(reps=1, fp8_js=FP8_JS, ht=HT, rcfg=RCFG, nch=NCH):
    """Mixed bf16 / fp8-DoubleRow kernel.

    Per j (Fourier g-pair), cos and sin planes live in one packed tile
    [128, 2(branch), ht+2, row] so a single Sin activation fills both.
    bf16 js issue 18 matmuls per strip (branch x tap); fp8 js issue
    9 * nch DoubleRow matmuls (branches are the two contraction
    sub-planes). Borders are pre-zeroed once per pool buffer; only image
    top/bottom edge rows are re-zeroed per edge chunk.

    rcfg selects the range-reduction pipeline: "tt" all-DVE fused rounds,
    "wrap" cos via ADD_RANGE_WRAP custom op, "wrapact" additionally moves
    the sin round onto the Scalar Engine (Identity + 2^23 bias).
    """
    bf_js = tuple(j for j in range(5) if j not in fp8_js)
    NTB = len(bf_js) * 2 * 9
    NT8 = len(fp8_js) * nch * 9
    nb = ht // 8
    RB = 80  # fp8 row stride (W+2=66 padded) for 16B-aligned plane strides
    RBB = W + 2  # bf16 row stride

    nc = bacc.Bacc("TRN2", target_bir_lowering=False)
    x_d = nc.dram_tensor("x", [BS, C, H, W], F32, kind="ExternalInput")
    if NTB:
        wb_d = nc.dram_tensor("wb", [128, NTB, 128], BF16, kind="ExternalInput")
    if NT8:
        w8_d = nc.dram_tensor("w8", [128, NT8, 2, 128], FP8, kind="ExternalInput")
    kv_d = nc.dram_tensor("kvec", [128, 5], F32, kind="ExternalInput")
    bias_d = nc.dram_tensor("biasv", [128, 1], F32, kind="ExternalInput")
    y_d = nc.dram_tensor("y", [BS, O, H, W], F32, kind="ExternalOutput")

    mult = mybir.AluOpType.mult
    add = mybir.AluOpType.add
    sub = mybir.AluOpType.subtract
    sin_f = mybir.ActivationFunctionType.Sin
    ident = mybir.ActivationFunctionType.Identity
    DR = mybir.MatmulPerfMode.DoubleRow

    with tile.TileContext(nc) as tc:
        with (
            tc.tile_pool(name="const", bufs=1) as cpool,
            tc.tile_pool(name="wpool", bufs=1) as wpool,
            tc.tile_pool(name="gen", bufs=2) as gen,
            tc.tile_pool(name="cspool", bufs=3) as cspool,
            tc.tile_pool(name="outp", bufs=3) as outp,
            tc.tile_pool(name="psum", bufs=2, space="PSUM") as psum,
        ):
            if NTB:
                wbt = wpool.tile([128, NTB, 128], BF16)
                for wi in range(0, NTB, 18):
                    nc.sync.dma_start(
                        wbt[:, wi : wi + 18, :], wb_d[:, wi : wi + 18, :]
                    )
            if NT8:
                w8t = wpool.tile([128, NT8, 2, 128], FP8)
                nc.sync.dma_start(w8t[:], w8_d[:])
            kvt = cpool.tile([128, 5], F32)
            nc.sync.dma_start(kvt[:], kv_d[:])
            bt = cpool.tile([128, 1], F32)
            nc.sync.dma_start(bt[:], bias_d[:])
            mg = cpool.tile([128, 1], F32)
            nc.vector.memset(mg[:], MAGIC)
            quarter = cpool.tile([128, 1], F32)
            nc.vector.memset(quarter[:], 0.25)

            # Pre-zero every cs-pool buffer once: column borders stay zero
            # forever (Sin only writes cols 1..64).
            pre8, preb = [], []
            for _ in range(3):
                if bf_js:
                    zb = cspool.tile([128, 2, ht + 2, RBB], BF16, tag="csb")
                    nc.vector.memset(zb[:], 0)
                    preb.append(zb)
                if fp8_js:
                    z8 = cspool.tile([128, 2, ht + 2, RB], FP8, tag="c8")
                    nc.gpsimd.memset(z8[:], 0)
                    pre8.append(z8)

            n_mm_groups = len(bf_js) * 2 * 9 + len(fp8_js) * nch * 9

            for rep in range(reps):
              for b in range(BS):
                for h0 in range(0, H, ht):
                    gr0, gr1 = max(0, h0 - 1), min(H, h0 + ht + 1)
                    l0 = gr0 - (h0 - 1)  # local row index of first real row
                    nrows = gr1 - gr0
                    rs = slice(l0, l0 + nrows)

                    xd = gen.tile([128, ht + 2, W], F32, tag="xdup")
                    nc.sync.dma_start(xd[0:64, rs, :], x_d[b, :, gr0:gr1, :])
                    nc.sync.dma_start(xd[64:128, rs, :], x_d[b, :, gr0:gr1, :])

                    pss = [
                        psum.tile([128, 8, 64], F32, tag=f"ps{bk}",
                                  name=f"ps{bk}_{rep}_{b}_{h0}")
                        for bk in range(nb)
                    ]

                    mm_idx = [0]  # running tap-group counter for start/stop

                    def gen_w2(j):
                        """Packed [128, 2, ht+2, W] tile of Sin args: plane 0
                        -> SGN_COS*cos(kx), plane 1 -> SGN_SIN*sin(kx)."""
                        us = gen.tile([128, ht + 2, W], F32, tag="us")
                        nc.vector.tensor_scalar(
                            us[:, rs, :], xd[:, rs, :],
                            kvt[:, j : j + 1], 16.0, mult, add,
                        )
                        w2 = gen.tile([128, 2, ht + 2, W], F32, tag="w2")
                        if rcfg in ("wrap", "wrapact"):
                            vs = gen.tile([128, ht + 2, W], F32, tag="vs")
                            if rcfg == "wrapact":
                                nc.scalar.activation(
                                    vs[:, rs, :], us[:, rs, :], ident,
                                    bias=mg[:],
                                )
                                nc.vector.scalar_tensor_tensor(
                                    w2[:, 1, rs, :], vs[:, rs, :], MAGIC,
                                    us[:, rs, :], sub, sub,
                                )
                            else:
                                nc.vector.tensor_scalar(
                                    vs[:, rs, :], us[:, rs, :], MAGIC, MAGIC,
                                    add, sub,
                                )
                                nc.vector.tensor_sub(
                                    w2[:, 1, rs, :], vs[:, rs, :], us[:, rs, :]
                                )
                            nc.vector.add_range_wrap(
                                w2[:, 0, rs, :], w2[:, 1, rs, :],
                                shift=0.25, bound=0.5, period=1.0,
                            )
                            return w2
                        # rcfg == "tt"
                        uc = gen.tile([128, ht + 2, W], F32, tag="uc")
                        nc.scalar.activation(
                            uc[:, rs, :], us[:, rs, :], ident, bias=quarter[:]
                        )
                        for u_t, pl in ((us, 1), (uc, 0)):
                            v_t = gen.tile([128, ht + 2, W], F32, tag="vt",
                                           bufs=1)
                            nc.vector.tensor_scalar(
                                v_t[:, rs, :], u_t[:, rs, :], MAGIC, MAGIC,
                                add, sub,
                            )
                            nc.vector.tensor_sub(
                                w2[:, pl, rs, :], u_t[:, rs, :], v_t[:, rs, :]
                            )
                        return w2

                    def fill_cs(j, cs, rb):
                        """Sin-fill the packed cs tile; zero edge rows."""
                        w2 = gen_w2(j)
                        if l0 == 1:
                            nc.gpsimd.memset(cs[:, :, 0:1, 0:66], 0)
                        if gr1 == H:
                            nc.gpsimd.memset(
                                cs[:, :, ht + 1 : ht + 2, 0:66], 0
                            )
                        nc.scalar.activation(
                            cs[:, :, rs, 1:65], w2[:, :, rs, :], sin_f,
                            scale=TWO_PI,
                        )

                    for j in bf_js:
                        jb = bf_js.index(j)
                        cs = cspool.tile([128, 2, ht + 2, RBB], BF16,
                                         tag="csb")
                        fill_cs(j, cs, RBB)
                        for br in range(2):
                            for dh in range(3):
                                for dw in range(3):
                                    t_idx = ((jb * 2 + br) * 3 + dh) * 3 + dw
                                    i0 = mm_idx[0]
                                    for bk in range(nb):
                                        nc.tensor.matmul(
                                            pss[bk][:],
                                            wbt[:, t_idx, :],
                                            cs[
                                                :, br,
                                                8 * bk + dh : 8 * bk + dh + 8,
                                                dw : dw + 64,
                                            ],
                                            start=(i0 == 0),
                                            stop=(i0 == n_mm_groups - 1),
                                        )
                                    mm_idx[0] += 1

                    for j in fp8_js:
                        j8 = fp8_js.index(j)
                        cs = cspool.tile([128, 2, ht + 2, RB], FP8, tag="c8")
                        fill_cs(j, cs, RB)
                        for ch in range(nch):
                            for dh in range(3):
                                for dw in range(3):
                                    t_idx = ((j8 * nch + ch) * 3 + dh) * 3 + dw
                                    i0 = mm_idx[0]
                                    for bk in range(nb):
                                        nc.tensor.matmul(
                                            pss[bk][:],
                                            w8t[:, t_idx, :, :],
                                            cs[
                                                :, :,
                                                8 * bk + dh : 8 * bk + dh + 8,
                                                dw : dw + 64,
                                            ],
                                            start=(i0 == 0),
                                            stop=(i0 == n_mm_groups - 1),
                                            perf_mode=DR,
                                        )
                                    mm_idx[0] += 1

                    assert mm_idx[0] == n_mm_groups

                    for bk in range(nb):
                        ob = outp.tile([128, 8, 64], F32, tag="ob")
                        nc.vector.tensor_scalar(
                            ob[:], pss[bk][:], 1.0 / WMAG, bt[:, 0:1],
                            mult, add,
                        )
                        nc.sync.dma_start(
                            y_d[b, :, h0 + 8 * bk : h0 + 8 * bk + 8, :], ob[:]
                        )
    nc.finalize()
    return nc


def _build_module(reps=1, mmdt="f32r", ht=HT):
    """Legacy single-dtype builder (f32r / bf16 / fp16)."""
    MMDT = {"f32r": F32R, "bf16": BF16, "fp16": mybir.dt.float16}[mmdt]
    nb = ht // 8
    nc = bacc.Bacc("TRN2", target_bir_lowering=False)
    x_d = nc.dram_tensor("x", [BS, C, H, W], F32, kind="ExternalInput")
    w_d = nc.dram_tensor("w", [128, NT, 128], MMDT, kind="ExternalInput")
    kv_d = nc.dram_tensor("kvec", [128, 5], F32, kind="ExternalInput")
    bias_d = nc.dram_tensor("biasv", [128, 1], F32, kind="ExternalInput")
    y_d = nc.dram_tensor("y", [BS, O, H, W], F32, kind="ExternalOutput")

    mult = mybir.AluOpType.mult
    add = mybir.AluOpType.add
    sin_f = mybir.ActivationFunctionType.Sin

    with tile.TileContext(nc) as tc:
        with (
            tc.tile_pool(name="const", bufs=1) as cpool,
            tc.tile_pool(name="wpool", bufs=1) as wpool,
            tc.tile_pool(name="gen", bufs=2) as gen,
            tc.tile_pool(name="cspool", bufs=3) as cspool,
            tc.tile_pool(name="outp", bufs=3) as outp,
            tc.tile_pool(name="psum", bufs=2, space="PSUM") as psum,
        ):
            wt = wpool.tile([128, NT, 128], MMDT)
            for wi in range(0, NT, 15):
                nc.sync.dma_start(
                    wt[:, wi : wi + 15, :], w_d[:, wi : wi + 15, :]
                )
            kvt = cpool.tile([128, 5], F32)
            nc.sync.dma_start(kvt[:], kv_d[:])
            bt = cpool.tile([128, 1], F32)
            nc.sync.dma_start(bt[:], bias_d[:])
            quarter = cpool.tile([128, 1], F32)
            nc.vector.memset(quarter[:], 0.25)

            for rep in range(reps):
              for b in range(BS):
                for h0 in range(0, H, ht):
                    gr0, gr1 = max(0, h0 - 1), min(H, h0 + ht + 1)
                    l0 = gr0 - (h0 - 1)  # local row index of first real row
                    nrows = gr1 - gr0
                    rs = slice(l0, l0 + nrows)

                    xd = gen.tile([128, ht + 2, W], F32, tag="xdup")
                    nc.sync.dma_start(xd[0:64, rs, :], x_d[b, :, gr0:gr1, :])
                    nc.sync.dma_start(xd[64:128, rs, :], x_d[b, :, gr0:gr1, :])

                    pss = [
                        psum.tile([128, 8, 64], F32, tag=f"ps{bk}",
                                  name=f"ps{bk}_{rep}_{b}_{h0}")
                        for bk in range(nb)
                    ]

                    for j in range(5):
                        # u = x*(k/2pi) + 16 ; v = round(u) ; w = u - v
                        us = gen.tile([128, ht + 2, W], F32, tag="us")
                        nc.vector.tensor_scalar(
                            us[:, rs, :], xd[:, rs, :],
                            kvt[:, j : j + 1], 16.0, mult, add,
                        )
                        uc = gen.tile([128, ht + 2, W], F32, tag="uc")
                        nc.scalar.activation(
                            uc[:, rs, :], us[:, rs, :],
                            mybir.ActivationFunctionType.Identity,
                            bias=quarter[:],
                        )

                        st = cspool.tile([128, ht + 2, W + 2], MMDT, tag="ss")
                        ct = cspool.tile([128, ht + 2, W + 2], MMDT, tag="cs")
                        for u_t, z in ((us, st), (uc, ct)):
                            v_t = gen.tile([128, ht + 2, W], F32, tag="vt", bufs=1)
                            nc.vector.tensor_scalar(
                                v_t[:, rs, :], u_t[:, rs, :], MAGIC, MAGIC,
                                mybir.AluOpType.add, mybir.AluOpType.subtract,
                            )
                            w_t = gen.tile([128, ht + 2, W], F32, tag="wt")
                            nc.vector.tensor_sub(
                                w_t[:, rs, :], u_t[:, rs, :], v_t[:, rs, :]
                            )
                            # zero borders (uint32 bitcast: memset can't
                            # encode fp32r), then fill interior with Sin
                            if mmdt == "f32r":
                                u32 = mybir.dt.uint32
                                zb = lambda ap: ap.bitcast(u32)
                            else:
                                zb = lambda ap: ap
                            nc.gpsimd.memset(zb(z[:, :, 0:1]), 0)
                            nc.gpsimd.memset(zb(z[:, :, W + 1 : W + 2]), 0)
                            if l0 == 1:
                                nc.gpsimd.memset(zb(z[:, 0:1, :]), 0)
                            if gr1 == H:
                                nc.gpsimd.memset(
                                    zb(z[:, ht + 1 : ht + 2, :]), 0
                                )
                            nc.scalar.activation(
                                z[:, rs, 1 : W + 1], w_t[:, rs, :], sin_f,
                                scale=TWO_PI,
                            )

                        for br in range(2):
                            src = ct if br == 0 else st
                            for dh in range(3):
                                for dw in range(3):
                                    t_idx = ((br * 5 + j) * 3 + dh) * 3 + dw
                                    for bk in range(nb):
                                        nc.tensor.matmul(
                                            pss[bk][:],
                                            wt[:, t_idx, :],
                                            src[
                                                :,
                                                8 * bk + dh : 8 * bk + dh + 8,
                                                dw : dw + 64,
                                            ],
                                            start=(j == 0 and br == 0
                                                   and dh == 0 and dw == 0),
                                            stop=(j == 4 and br == 1
                                                  and dh == 2 and dw == 2),
                                        )

                    for bk in range(nb):
                        ob = outp.tile([128, 8, 64], F32, tag="ob")
                        nc.vector.tensor_scalar_add(ob[:], pss[bk][:], bt[:, 0:1])
                        nc.sync.dma_start(
                            y_d[b, :, h0 + 8 * bk : h0 + 8 * bk + 8, :], ob[:]
                        )
    nc.finalize()
    return nc


def _get_module(reps=1, mmdt="mix", ht=HT, fp8_js=FP8_JS, rcfg=RCFG, nch=NCH):
    if mmdt == "mix":
        key = ("mix", reps, ht, fp8_js, rcfg, nch)
        if key not in _CACHE:
            _CACHE[key] = _build_mixed(reps, fp8_js, ht, rcfg, nch)
        return _CACHE[key]
    key = ("nc", reps, mmdt, ht)
    if key not in _CACHE:
        _CACHE[key] = _build_module(reps, mmdt, ht)
    return _CACHE[key]


def _np_mmdt(mmdt):
    import ml_dtypes
    return {"f32r": np.float32, "bf16": ml_dtypes.bfloat16,
            "fp16": np.float16}[mmdt]


def _weight_planes(fc):
    # fc: (2, O, C, kH, kW, G) -> (br, j, kh, kw, p=(gp*64+c), o) fp32
    W6 = np.transpose(fc, (0, 5, 3, 4, 2, 1))  # (br, g, kh, kw, c, o)
    W6 = W6.reshape(2, 5, 2, 3, 3, 64, 128)  # (br, j, gp, kh, kw, c, o)
    Wt = np.transpose(W6, (0, 1, 3, 4, 2, 5, 6))  # (br, j, kh, kw, gp, c, o)
    return Wt.reshape(2, 5, 3, 3, 128, 128)


def _host_weights(fc, mmdt="f32r"):
    # legacy single-dtype layout: w[p, t=(br,j,kh,kw), o]
    Wt = _weight_planes(fc).reshape(NT, 128, 128)
    return np.ascontiguousarray(
        np.transpose(Wt, (1, 0, 2)).astype(_np_mmdt(mmdt))
    )


def _host_weights_mixed(fc, fp8_js=FP8_JS, rcfg=RCFG, nch=NCH):
    import ml_dtypes
    sc, ss = _SIGNS[rcfg]
    Wp = _weight_planes(fc) * WMAG  # (br, j, kh, kw, p, o)
    Wp = Wp * np.array([sc, ss]).reshape(2, 1, 1, 1, 1, 1)
    bf_js = tuple(j for j in range(5) if j not in fp8_js)
    out = {}
    if bf_js:
        wb = Wp[:, bf_js]  # (br, nj, kh, kw, p, o)
        # tile order: ((jb*2 + br)*3 + kh)*3 + kw
        wb = np.transpose(wb, (1, 0, 2, 3, 4, 5))  # (nj, br, kh, kw, p, o)
        wb = wb.reshape(len(bf_js) * 2 * 9, 128, 128)
        out["wb"] = np.ascontiguousarray(
            np.transpose(wb, (1, 0, 2)).astype(ml_dtypes.bfloat16)
        )
    if fp8_js:
        w8 = Wp[:, fp8_js]  # (br, nj, kh, kw, p, o)
        w8 = np.transpose(w8, (1, 2, 3, 4, 0, 5))  # (nj, kh, kw, p, br, o)
        w8 = w8.astype(np.float32)
        hi = w8.astype(ml_dtypes.float8_e4m3)
        if nch == 2:
            lo = (w8 - hi.astype(np.float32)).astype(ml_dtypes.float8_e4m3)
            w8q = np.stack([hi, lo], axis=1)  # (nj, nch, kh, kw, p, br, o)
        else:
            w8q = hi[:, None]
        w8q = w8q.reshape(len(fp8_js) * nch * 9, 128, 2, 128)
        out["w8"] = np.ascontiguousarray(np.transpose(w8q, (1, 0, 2, 3)))
    return out


def _host_kvec():
    kvec = np.zeros((128, 5), np.float32)
    for j in range(5):
        kvec[0:64, j] = (2 * j + 1) / TWO_PI
        kvec[64:128, j] = (2 * j + 2) / TWO_PI
    return kvec


def _host_inputs(x, fouriercoeffs, bias, mmdt="mix", fp8_js=FP8_JS, rcfg=RCFG):
    x = np.ascontiguousarray(np.asarray(x, dtype=np.float32))
    fc = np.asarray(fouriercoeffs, dtype=np.float32)
    base = {
        "kvec": _host_kvec(),
        "biasv": np.ascontiguousarray(
            np.asarray(bias, dtype=np.float32).reshape(128, 1)
        ),
    }
    if mmdt == "mix":
        base.update(_host_weights_mixed(fc, fp8_js, rcfg))
    else:
        base["w"] = _host_weights(fc, mmdt)
    return x, base


def kernel(x, fouriercoeffs, bias):
    x, base = _host_inputs(x, fouriercoeffs, bias, "mix", FP8_JS, RCFG)
    nc = _get_module(1, "mix", fp8_js=FP8_JS, rcfg=RCFG)
    in_maps = [
        dict(base, x=x[i * BS : (i + 1) * BS]) for i in range(N_CORES)
    ]
    res = run_bass_kernel_spmd(nc, in_maps, list(range(N_CORES))).results
    return np.concatenate([res[i]["y"] for i in range(N_CORES)], axis=0)
